# revision 12
# baseline (speedup 1.0000x reference)
"""2-layer IndRNN (diagonal recurrence) + linear head on 8 trn2 NeuronCores.

Data-parallel over batch: 32 rows/core, all 32 in the free dim (no chunk
split; free size 512 per op).

Numerics (validated ~1.45e-2 rel err vs fp64, gate 2e-2):
  - GEMM-0 (x @ W0^T) in fp16, 1 cyc/row. PSUM f32, drained with bias to an
    fp16 pre/z ring on GpSimd (tensor_scalar_add).
  - Recurrences keep fp16 PRE-activation state z_t in place in the ring;
    each step is stt((z_{t-1} max 0) mult u[f32]) + fp16 tensor_add (DVE 2x).
  - h0 = relu(z0) emitted blockwise by ACT as fp8e4m3 scaled by 2^4
    (exact power-2 folding), per m0-pair so GEMM-1 can start early.
  - GEMM-1 (h0 @ W1^T) in fp8e4m3 DoubleRow: weights scaled by 2^13, 8
    k-pair matmuls (256-deep each) per m-tile -> 2x bf16 throughput. The
    PSUM drain applies scale 2^-17 and bias on ACT, writing the fp16 rb
    ring consumed in-place by recurrence 1.
  - Head: relu(z1_99) fp16, 16-step accumulated [128,1]x[128,32] matmul,
    + lin_b on the final ACT copy.
Host side only reorders/converts numpy inputs; all FLOPs run on device.
"""

import numpy as np

B, T, I, H = 256, 100, 128, 2048
NCORES = 8
BL = B // NCORES            # 32 batch rows per core, all in free dim
NO = H // 128               # 16 hidden tiles
KP = NO // 2                # 8 DoubleRow k-pairs
SH = 8.0                    # h0 fp8 scale (power of 2); kept low because the
                            # fp8 matmul path NaNs when |PSUM| nears fp16 max
TBLKS = [(0, 16), (16, 16), (32, 16), (48, 16), (64, 16), (80, 16), (96, 4)]

_CACHE = {}


def _build(sw_scale):
    import concourse.tile as tile
    from concourse import bacc, mybir

    f32 = mybir.dt.float32
    f16 = mybir.dt.float16
    f8 = mybir.dt.float8e4
    RELU = mybir.ActivationFunctionType.Relu
    IDENT = mybir.ActivationFunctionType.Identity
    MAX = mybir.AluOpType.max
    MULT = mybir.AluOpType.mult
    DR = mybir.MatmulPerfMode.DoubleRow
    SC = 1.0 / (sw_scale * SH)  # GEMM-1 drain descale (exact power of 2)

    nc = bacc.Bacc(None, target_bir_lowering=False)

    xT_d = nc.dram_tensor("xT", [128, T, BL], f16, kind="ExternalInput")
    w0T_d = nc.dram_tensor("w0T", [128, NO, 128], f16, kind="ExternalInput")
    w1T_d = nc.dram_tensor("w1T", [128, KP, 2, NO, 128], f8, kind="ExternalInput")
    u0f_d = nc.dram_tensor("u0f", [128, NO, BL], f32, kind="ExternalInput")
    u1f_d = nc.dram_tensor("u1f", [128, NO, BL], f32, kind="ExternalInput")
    b0_d = nc.dram_tensor("b0t", [128, NO], f32, kind="ExternalInput")
    b1_d = nc.dram_tensor("b1t", [128, NO], f32, kind="ExternalInput")
    lw_d = nc.dram_tensor("lwt", [128, NO], f16, kind="ExternalInput")
    lb_d = nc.dram_tensor("lbt", [1, 1], f32, kind="ExternalInput")
    out_d = nc.dram_tensor("out", [1, BL], f32, kind="ExternalOutput")

    NB = len(TBLKS)

    with tile.TileContext(nc) as tc:
        with (
            tc.tile_pool(name="const", bufs=1) as const,
            tc.tile_pool(name="pb", bufs=3) as pbp,
            tc.tile_pool(name="hb", bufs=2) as hbp,
            tc.tile_pool(name="rb", bufs=3) as rbp,
            tc.tile_pool(name="tm", bufs=6) as tmp,
            tc.tile_pool(name="ps0", bufs=3, space="PSUM") as ps0,
            tc.tile_pool(name="ps1", bufs=3, space="PSUM") as ps1,
        ):
            xt = const.tile([128, T, BL], f16, tag="xt")
            w0T = const.tile([128, NO, 128], f16, tag="w0T")
            w1T = const.tile([128, KP, 2, NO, 128], f8, tag="w1T")
            u0f = const.tile([128, NO, BL], f32, tag="u0f")
            u1f = const.tile([128, NO, BL], f32, tag="u1f")
            b0t = const.tile([128, NO], f32, tag="b0t")
            b1t = const.tile([128, NO], f32, tag="b1t")
            lwt = const.tile([128, NO], f16, tag="lwt")
            lbt = const.tile([1, 1], f32, tag="lbt")
            outs = const.tile([1, BL], f32, tag="outs")
            h1h = const.tile([128, NO, BL], f16, tag="h1h")

            nc.sync.dma_start(out=w0T[:], in_=w0T_d[:])
            nc.sync.dma_start(out=xt[:], in_=xT_d[:])
            nc.sync.dma_start(out=u0f[:], in_=u0f_d[:])
            nc.sync.dma_start(out=u1f[:], in_=u1f_d[:])
            nc.sync.dma_start(out=b0t[:], in_=b0_d[:])
            nc.sync.dma_start(out=b1t[:], in_=b1_d[:])
            nc.sync.dma_start(out=lwt[:], in_=lw_d[:])
            nc.sync.dma_start(out=lbt[:], in_=lb_d[:])

            pbs, hbs, rbs = {}, {}, {}

            def new_pb(b):
                pb = pbp.tile([128, NO, 16, BL], f16, tag="pb")
                pbs[b] = pb

            def new_hb(b):
                hb = hbp.tile([128, NO, 16, BL], f8, tag="hb")
                hbs[b] = hb

            def new_rb(b):
                rb = rbp.tile([128, NO, 16, BL], f16, tag="rb")
                rbs[b] = rb

            def g0_mm(b, m0):
                t0, TB = TBLKS[b]
                ps = ps0.tile([128, 16, BL], f32, tag="ps0")
                nc.tensor.matmul(
                    ps[:, :TB], w0T[:, m0], xt[:, t0:t0 + TB],
                    start=True, stop=True,
                )
                return ps

            def g0_drain(b, m0, ps, on_act):
                t0, TB = TBLKS[b]
                pb = pbs[b]
                if on_act:
                    nc.scalar.activation(
                        pb[:, m0, :TB], ps[:, :TB], IDENT,
                        bias=b0t[:, m0:m0 + 1], scale=1.0,
                    )
                else:
                    nc.vector.tensor_scalar_add(
                        pb[:, m0, :TB], ps[:, :TB], b0t[:, m0:m0 + 1],
                    )

            def r_chain(which, b, alternate):
                blks, uf = (pbs, u0f) if which == 0 else (rbs, u1f)
                blk = blks[b]
                t0, TB = TBLKS[b]
                for trel in range(TB):
                    if t0 + trel == 0:
                        continue
                    prev = (blk[:, :, trel - 1] if trel
                            else blks[b - 1][:, :, TBLKS[b - 1][1] - 1])
                    tm = tmp.tile([128, NO, BL], f16, tag="tm")
                    nc.vector.scalar_tensor_tensor(
                        tm[:], prev, 0.0, uf[:], MAX, MULT,
                    )
                    cur = blk[:, :, trel]
                    eng = nc.gpsimd if (alternate and (trel & 1)) else nc.vector
                    eng.tensor_add(cur, tm[:], cur)

            def h0_pair(b, kp):
                t0, TB = TBLKS[b]
                nc.scalar.activation(
                    hbs[b][:, 2 * kp:2 * kp + 2, :TB],
                    pbs[b][:, 2 * kp:2 * kp + 2, :TB],
                    RELU, scale=SH,
                )

            def g1_group(b, m):
                t0, TB = TBLKS[b]
                ps = ps1.tile([128, 16, BL], f32, tag="ps1")
                for kp in range(KP):
                    nc.tensor.matmul(
                        ps[:, :TB],
                        w1T[:, kp, :, m],
                        hbs[b][:, 2 * kp:2 * kp + 2, :TB],
                        start=(kp == 0), stop=(kp == KP - 1),
                        perf_mode=DR,
                    )
                nc.scalar.activation(
                    rbs[b][:, m, :TB], ps[:, :TB], IDENT,
                    bias=b1t[:, m:m + 1], scale=SC,
                )

            def head():
                lt0, lTB = TBLKS[NB - 1]
                nc.scalar.activation(
                    h1h[:], rbs[NB - 1][:, :, lTB - 1], RELU, scale=1.0,
                )
                ph = ps0.tile([128, 16, BL], f32, tag="ps0")
                for m in range(NO):
                    nc.tensor.matmul(
                        ph[0:1, 0], lwt[:, m:m + 1], h1h[:, m],
                        start=(m == 0), stop=(m == NO - 1),
                    )
                nc.scalar.activation(
                    outs[0:1, :], ph[0:1, 0], IDENT,
                    bias=lbt[0:1, 0:1], scale=1.0,
                )

            # ---- software pipeline ----
            # Steady-state slot b: PE runs g1(b) groups with g0(b+2)
            # matmuls interleaved (keeps ps0 drains spaced so PE never
            # stalls on bank recycling); ACT runs drain1(b,m), drain0(b+2,m)
            # and the h0(b+1) pairs in the gaps; DVE runs the r1(b-1) then
            # r0(b+2) chains with odd-step adds on GpSimd (except in tail
            # blocks where g0 is finished and DVE has slack).
            new_pb(0)
            for m0 in range(NO):
                ps = g0_mm(0, m0)
                g0_drain(0, m0, ps, on_act=(m0 % 2 == 0))
            for kp in range(KP):
                nc.sync.dma_start(out=w1T[:, kp], in_=w1T_d[:, kp])
            r_chain(0, 0, alternate=False)
            new_pb(1)
            for m0 in range(NO):
                ps = g0_mm(1, m0)
                g0_drain(1, m0, ps, on_act=True)
            new_hb(0)
            for kp in range(KP):
                h0_pair(0, kp)
            r_chain(0, 1, alternate=True)
            for b in range(NB):
                new_rb(b)
                if b + 2 < NB:
                    new_pb(b + 2)
                if b + 1 < NB:
                    new_hb(b + 1)
                for m in range(NO):
                    g1_group(b, m)
                    if b + 2 < NB:
                        ps = g0_mm(b + 2, m)
                        g0_drain(b + 2, m, ps, on_act=True)
                    if (m % 2 == 1) and (b + 1 < NB):
                        h0_pair(b + 1, m // 2)
                if b >= 1:
                    r_chain(1, b - 1, alternate=(b - 1 < NB - 2))
                if b + 2 < NB:
                    r_chain(0, b + 2, alternate=(b + 2 < NB - 1))
            r_chain(1, NB - 1, alternate=False)
            head()

            nc.sync.dma_start(out=out_d[:], in_=outs[:])

    nc.compile()
    return nc


def _get_nc(sw_scale=8192.0):
    key = ("nc", sw_scale)
    if key not in _CACHE:
        _CACHE[key] = _build(sw_scale)
    return _CACHE[key]


def _prep_shared(W0, b0, u0, W1, b1, u1, lin_w, lin_b):
    import ml_dtypes

    # power-of-2 weight scale keeping max|W1|*sw < 448 (e4m3 max)
    wmax = float(np.abs(W1).max()) or 1.0
    sw = float(2.0 ** np.floor(np.log2(448.0 / wmax)))
    sw = min(sw, 2048.0)

    w0T = np.ascontiguousarray(W0.T.reshape(128, NO, 128)).astype(np.float16)
    w1T = np.ascontiguousarray(
        W1.reshape(NO, 128, KP, 2, 128).transpose(4, 2, 3, 0, 1) * sw
    ).astype(ml_dtypes.float8_e4m3fn)
    u0f = np.ascontiguousarray(
        np.broadcast_to(u0.reshape(NO, 128).T[:, :, None], (128, NO, BL))
    ).astype(np.float32)
    u1f = np.ascontiguousarray(
        np.broadcast_to(u1.reshape(NO, 128).T[:, :, None], (128, NO, BL))
    ).astype(np.float32)
    b0t = np.ascontiguousarray(b0.reshape(NO, 128).T).astype(np.float32)
    b1t = np.ascontiguousarray(b1.reshape(NO, 128).T).astype(np.float32)
    lwt = np.ascontiguousarray(lin_w.reshape(NO, 128).T).astype(np.float16)
    lbt = np.ascontiguousarray(lin_b.reshape(1, 1)).astype(np.float32)
    return sw, dict(w0T=w0T, w1T=w1T, u0f=u0f, u1f=u1f,
                    b0t=b0t, b1t=b1t, lwt=lwt, lbt=lbt)


def make_in_maps(x, W0, b0, u0, W1, b1, u1, lin_w, lin_b):
    sw, shared = _prep_shared(
        np.asarray(W0, np.float32), np.asarray(b0, np.float32),
        np.asarray(u0, np.float32), np.asarray(W1, np.float32),
        np.asarray(b1, np.float32), np.asarray(u1, np.float32),
        np.asarray(lin_w, np.float32), np.asarray(lin_b, np.float32),
    )
    x = np.asarray(x, np.float32)
    in_maps = []
    for core in range(NCORES):
        xc = x[core * BL:(core + 1) * BL]            # (BL, T, I)
        xT = np.ascontiguousarray(xc.transpose(2, 1, 0)).astype(np.float16)
        in_maps.append({"xT": xT, **shared})
    return sw, in_maps


def kernel(x, W0, b0, u0, W1, b1, u1, lin_w, lin_b):
    from concourse.bass_utils import run_bass_kernel_spmd

    sw, in_maps = make_in_maps(x, W0, b0, u0, W1, b1, u1, lin_w, lin_b)
    nc = _get_nc(sw)
    try:
        res = run_bass_kernel_spmd(nc, in_maps, list(range(NCORES)))
    except Exception:
        res = run_bass_kernel_spmd(nc, in_maps, list(range(NCORES)))
    return np.concatenate([r["out"][0] for r in res.results])


# revision 13
# speedup vs baseline: 1.1796x; 1.1796x over previous
"""2-layer IndRNN (diagonal recurrence) + linear head on 8 trn2 NeuronCores.

Data-parallel over batch: 32 rows/core, all 32 in the free dim (no chunk
split; free size 512 per op).

Numerics (validated ~1.45e-2 rel err vs fp64, gate 2e-2):
  - GEMM-0 (x @ W0^T) in fp16, 1 cyc/row. PSUM f32, drained with bias to an
    fp16 pre/z ring on GpSimd (tensor_scalar_add).
  - Recurrences keep fp16 PRE-activation state z_t in place in the ring;
    each step is stt((z_{t-1} max 0) mult u[f32]) + fp16 tensor_add (DVE 2x).
  - h0 = relu(z0) emitted blockwise by ACT as fp8e4m3 scaled by 2^4
    (exact power-2 folding), per m0-pair so GEMM-1 can start early.
  - GEMM-1 (h0 @ W1^T) in fp8e4m3 DoubleRow: weights scaled by 2^13, 8
    k-pair matmuls (256-deep each) per m-tile -> 2x bf16 throughput. The
    PSUM drain applies scale 2^-17 and bias on ACT, writing the fp16 rb
    ring consumed in-place by recurrence 1.
  - Head: relu(z1_99) fp16, 16-step accumulated [128,1]x[128,32] matmul,
    + lin_b on the final ACT copy.
Host side only reorders/converts numpy inputs; all FLOPs run on device.
"""

import numpy as np

B, T, I, H = 256, 100, 128, 2048
NCORES = 8
BL = B // NCORES            # 32 batch rows per core, all in free dim
NO = H // 128               # 16 hidden tiles
KP = NO // 2                # 8 DoubleRow k-pairs
SH = 8.0                    # h0 fp8 scale (power of 2); kept low because the
                            # fp8 matmul path NaNs when |PSUM| nears fp16 max
TBLKS = [(0, 16), (16, 16), (32, 16), (48, 16), (64, 16), (80, 16), (96, 4)]

_CACHE = {}


def _build(sw_scale):
    import concourse.tile as tile
    from concourse import bacc, mybir

    f32 = mybir.dt.float32
    f16 = mybir.dt.float16
    f8 = mybir.dt.float8e4
    RELU = mybir.ActivationFunctionType.Relu
    IDENT = mybir.ActivationFunctionType.Identity
    MAX = mybir.AluOpType.max
    MULT = mybir.AluOpType.mult
    DR = mybir.MatmulPerfMode.DoubleRow
    SC = 1.0 / (sw_scale * SH)  # GEMM-1 drain descale (exact power of 2)

    nc = bacc.Bacc(None, target_bir_lowering=False)

    xT_d = nc.dram_tensor("xT", [128, T, BL], f16, kind="ExternalInput")
    w0T_d = nc.dram_tensor("w0T", [128, NO, 128], f16, kind="ExternalInput")
    w1T_d = nc.dram_tensor("w1T", [128, KP, 2, NO, 128], f8, kind="ExternalInput")
    u0f_d = nc.dram_tensor("u0f", [128, NO, BL], f32, kind="ExternalInput")
    u1f_d = nc.dram_tensor("u1f", [128, NO, BL], f32, kind="ExternalInput")
    b0_d = nc.dram_tensor("b0t", [128, NO], f32, kind="ExternalInput")
    b1_d = nc.dram_tensor("b1t", [128, NO], f32, kind="ExternalInput")
    lw_d = nc.dram_tensor("lwt", [128, NO], f16, kind="ExternalInput")
    lb_d = nc.dram_tensor("lbt", [1, 1], f32, kind="ExternalInput")
    out_d = nc.dram_tensor("out", [1, BL], f32, kind="ExternalOutput")

    NB = len(TBLKS)

    with tile.TileContext(nc) as tc:
        with (
            tc.tile_pool(name="const", bufs=1) as const,
            tc.tile_pool(name="pb", bufs=3) as pbp,
            tc.tile_pool(name="hb", bufs=2) as hbp,
            tc.tile_pool(name="rb", bufs=3) as rbp,
            tc.tile_pool(name="tm", bufs=6) as tmp,
            tc.tile_pool(name="ps0", bufs=3, space="PSUM") as ps0,
            tc.tile_pool(name="ps1", bufs=3, space="PSUM") as ps1,
        ):
            xt = const.tile([128, T, BL], f16, tag="xt")
            w0T = const.tile([128, NO, 128], f16, tag="w0T")
            w1T = const.tile([128, KP, 2, NO, 128], f8, tag="w1T")
            u0f = const.tile([128, NO, BL], f32, tag="u0f")
            u1f = const.tile([128, NO, BL], f32, tag="u1f")
            b0t = const.tile([128, NO], f32, tag="b0t")
            b1t = const.tile([128, NO], f32, tag="b1t")
            lwt = const.tile([128, NO], f16, tag="lwt")
            lbt = const.tile([1, 1], f32, tag="lbt")
            outs = const.tile([1, BL], f32, tag="outs")
            h1h = const.tile([128, NO, BL], f16, tag="h1h")

            nc.sync.dma_start(out=w0T[:], in_=w0T_d[:])
            nc.sync.dma_start(out=xt[:], in_=xT_d[:])
            nc.sync.dma_start(out=u0f[:], in_=u0f_d[:])
            nc.sync.dma_start(out=u1f[:], in_=u1f_d[:])
            nc.sync.dma_start(out=b0t[:], in_=b0_d[:])
            nc.sync.dma_start(out=b1t[:], in_=b1_d[:])
            nc.sync.dma_start(out=lwt[:], in_=lw_d[:])
            nc.sync.dma_start(out=lbt[:], in_=lb_d[:])

            pbs, hbs, rbs = {}, {}, {}

            def new_pb(b):
                pb = pbp.tile([128, NO, 16, BL], f16, tag="pb")
                pbs[b] = pb

            def new_hb(b):
                hb = hbp.tile([128, NO, 16, BL], f8, tag="hb")
                hbs[b] = hb

            def new_rb(b):
                rb = rbp.tile([128, NO, 16, BL], f16, tag="rb")
                rbs[b] = rb

            def g0_mm(b, m0):
                t0, TB = TBLKS[b]
                ps = ps0.tile([128, 16, BL], f32, tag="ps0")
                nc.tensor.matmul(
                    ps[:, :TB], w0T[:, m0], xt[:, t0:t0 + TB],
                    start=True, stop=True,
                )
                return ps

            def g0_drain(b, m0, ps, on_act):
                t0, TB = TBLKS[b]
                pb = pbs[b]
                if on_act:
                    nc.scalar.activation(
                        pb[:, m0, :TB], ps[:, :TB], IDENT,
                        bias=b0t[:, m0:m0 + 1], scale=1.0,
                    )
                else:
                    nc.vector.tensor_scalar_add(
                        pb[:, m0, :TB], ps[:, :TB], b0t[:, m0:m0 + 1],
                    )

            def r_chain(which, b, alternate):
                blks, uf = (pbs, u0f) if which == 0 else (rbs, u1f)
                blk = blks[b]
                t0, TB = TBLKS[b]
                for trel in range(TB):
                    if t0 + trel == 0:
                        continue
                    prev = (blk[:, :, trel - 1] if trel
                            else blks[b - 1][:, :, TBLKS[b - 1][1] - 1])
                    tm = tmp.tile([128, NO, BL], f16, tag="tm")
                    nc.vector.scalar_tensor_tensor(
                        tm[:], prev, 0.0, uf[:], MAX, MULT,
                    )
                    cur = blk[:, :, trel]
                    eng = nc.gpsimd if (alternate and (trel & 1)) else nc.vector
                    eng.tensor_add(cur, tm[:], cur)

            def h0_pair(b, kp):
                t0, TB = TBLKS[b]
                nc.scalar.activation(
                    hbs[b][:, 2 * kp:2 * kp + 2, :TB],
                    pbs[b][:, 2 * kp:2 * kp + 2, :TB],
                    RELU, scale=SH,
                )

            def g1_group(b, m):
                t0, TB = TBLKS[b]
                ps = ps1.tile([128, 16, BL], f32, tag="ps1")
                for kp in range(KP):
                    nc.tensor.matmul(
                        ps[:, :TB],
                        w1T[:, kp, :, m],
                        hbs[b][:, 2 * kp:2 * kp + 2, :TB],
                        start=(kp == 0), stop=(kp == KP - 1),
                        perf_mode=DR,
                    )
                nc.scalar.activation(
                    rbs[b][:, m, :TB], ps[:, :TB], IDENT,
                    bias=b1t[:, m:m + 1], scale=SC,
                )

            def head():
                lt0, lTB = TBLKS[NB - 1]
                nc.scalar.activation(
                    h1h[:], rbs[NB - 1][:, :, lTB - 1], RELU, scale=1.0,
                )
                ph = ps0.tile([128, 16, BL], f32, tag="ps0")
                for m in range(NO):
                    nc.tensor.matmul(
                        ph[0:1, 0], lwt[:, m:m + 1], h1h[:, m],
                        start=(m == 0), stop=(m == NO - 1),
                    )
                nc.scalar.activation(
                    outs[0:1, :], ph[0:1, 0], IDENT,
                    bias=lbt[0:1, 0:1], scale=1.0,
                )

            # ---- software pipeline ----
            # Steady-state slot b: PE runs g1(b) groups with g0(b+2)
            # matmuls interleaved (keeps ps0 drains spaced so PE never
            # stalls on bank recycling); ACT runs drain1(b,m), drain0(b+2,m)
            # and the h0(b+1) pairs in the gaps; DVE runs the r1(b-1) then
            # r0(b+2) chains with odd-step adds on GpSimd (except in tail
            # blocks where g0 is finished and DVE has slack).
            new_pb(0)
            for m0 in range(NO):
                ps = g0_mm(0, m0)
                g0_drain(0, m0, ps, on_act=(m0 % 2 == 0))
            for kp in range(KP):
                nc.sync.dma_start(out=w1T[:, kp], in_=w1T_d[:, kp])
            r_chain(0, 0, alternate=False)
            new_pb(1)
            for m0 in range(NO):
                ps = g0_mm(1, m0)
                g0_drain(1, m0, ps, on_act=True)
            new_hb(0)
            for kp in range(KP):
                h0_pair(0, kp)
            r_chain(0, 1, alternate=False)
            for b in range(NB):
                new_rb(b)
                if b + 2 < NB:
                    new_pb(b + 2)
                if b + 1 < NB:
                    new_hb(b + 1)
                for m in range(NO):
                    g1_group(b, m)
                    if b + 2 < NB:
                        ps = g0_mm(b + 2, m)
                        g0_drain(b + 2, m, ps, on_act=True)
                    if (m % 2 == 1) and (b + 1 < NB):
                        h0_pair(b + 1, m // 2)
                if b >= 1:
                    r_chain(1, b - 1, alternate=False)
                if b + 2 < NB:
                    r_chain(0, b + 2, alternate=False)
            r_chain(1, NB - 1, alternate=False)
            head()

            nc.sync.dma_start(out=out_d[:], in_=outs[:])

    nc.compile()
    return nc


def _get_nc(sw_scale=8192.0):
    key = ("nc", sw_scale)
    if key not in _CACHE:
        _CACHE[key] = _build(sw_scale)
    return _CACHE[key]


def _prep_shared(W0, b0, u0, W1, b1, u1, lin_w, lin_b):
    import ml_dtypes

    # power-of-2 weight scale keeping max|W1|*sw < 448 (e4m3 max)
    wmax = float(np.abs(W1).max()) or 1.0
    sw = float(2.0 ** np.floor(np.log2(448.0 / wmax)))
    sw = min(sw, 2048.0)

    w0T = np.ascontiguousarray(W0.T.reshape(128, NO, 128)).astype(np.float16)
    w1T = np.ascontiguousarray(
        W1.reshape(NO, 128, KP, 2, 128).transpose(4, 2, 3, 0, 1) * sw
    ).astype(ml_dtypes.float8_e4m3fn)
    u0f = np.ascontiguousarray(
        np.broadcast_to(u0.reshape(NO, 128).T[:, :, None], (128, NO, BL))
    ).astype(np.float32)
    u1f = np.ascontiguousarray(
        np.broadcast_to(u1.reshape(NO, 128).T[:, :, None], (128, NO, BL))
    ).astype(np.float32)
    b0t = np.ascontiguousarray(b0.reshape(NO, 128).T).astype(np.float32)
    b1t = np.ascontiguousarray(b1.reshape(NO, 128).T).astype(np.float32)
    lwt = np.ascontiguousarray(lin_w.reshape(NO, 128).T).astype(np.float16)
    lbt = np.ascontiguousarray(lin_b.reshape(1, 1)).astype(np.float32)
    return sw, dict(w0T=w0T, w1T=w1T, u0f=u0f, u1f=u1f,
                    b0t=b0t, b1t=b1t, lwt=lwt, lbt=lbt)


def make_in_maps(x, W0, b0, u0, W1, b1, u1, lin_w, lin_b):
    sw, shared = _prep_shared(
        np.asarray(W0, np.float32), np.asarray(b0, np.float32),
        np.asarray(u0, np.float32), np.asarray(W1, np.float32),
        np.asarray(b1, np.float32), np.asarray(u1, np.float32),
        np.asarray(lin_w, np.float32), np.asarray(lin_b, np.float32),
    )
    x = np.asarray(x, np.float32)
    in_maps = []
    for core in range(NCORES):
        xc = x[core * BL:(core + 1) * BL]            # (BL, T, I)
        xT = np.ascontiguousarray(xc.transpose(2, 1, 0)).astype(np.float16)
        in_maps.append({"xT": xT, **shared})
    return sw, in_maps


def kernel(x, W0, b0, u0, W1, b1, u1, lin_w, lin_b):
    from concourse.bass_utils import run_bass_kernel_spmd

    sw, in_maps = make_in_maps(x, W0, b0, u0, W1, b1, u1, lin_w, lin_b)
    nc = _get_nc(sw)
    try:
        res = run_bass_kernel_spmd(nc, in_maps, list(range(NCORES)))
    except Exception:
        res = run_bass_kernel_spmd(nc, in_maps, list(range(NCORES)))
    return np.concatenate([r["out"][0] for r in res.results])
